# revision 15
# baseline (speedup 1.0000x reference)
"""Trainium2 Bass kernel for nn_CLCRNModel (CLCRN encoder-decoder GNN).

Strategy: data-parallel over batch (8 batch elements -> 8 NeuronCores).
The sparse 25-neighbor graph conv is cast as a dense matmul against the
(2048, 2048) row-normalized adjacency A kept resident in SBUF (bf16).
Per GRU cell: 4 chained A-passes on the PE (gate 2-hop + candidate 2-hop)
plus small dense weight matmuls in fp32r. Activations flow in two layouts:
channel-major (C, N) for dense-W rhs / elementwise, and natural
node-major (128, C) tiles (bf16) as matmul stationary operands, produced
by PE transposes.
"""
import os
import sys

for _p in ("/root/.axon_site/_ro/trn_rl_repo", "/opt/trn_rl_repo"):
    if os.path.isdir(_p) and _p not in sys.path:
        sys.path.append(_p)

import numpy as np
import ml_dtypes

import concourse.bass as bass
import concourse.mybir as mybir
import concourse.tile as tile
from concourse.bass_utils import run_bass_kernel_spmd
from concourse.masks import make_identity

P = 128
N = 2048
NT = 16            # node tiles
S = 12             # encoder steps
HOR = 12           # decoder steps
EMB = 16
H = 64             # GRU units
CX = 33            # encoder x-part channels [feat16 | x1 | node16]
ZE = 97            # encoder z channels (CX + H)
ZD = 65            # decoder z channels (1 + H)
FREE = 512
NCH = N // FREE    # 4 free chunks
NCORES = 8

F32 = mybir.dt.float32
F32R = mybir.dt.float32r
BF16 = mybir.dt.bfloat16
FP8 = mybir.dt.float8e4
AF = mybir.ActivationFunctionType


def _r(ap):
    return ap.bitcast(F32R)


def _split_multiwait(nc, max_waits=1):
    """This container's walrus rejects >1 sem-wait on CTRL-class
    instructions (the Tile exit drain carries one wait per live sem).
    Split excess waits onto preceding same-engine carrier drains."""
    fn = nc.m.functions[0]
    n = 0
    for blk in fn.blocks:
        out = []
        for ins in blk.instructions:
            si = ins.sync_info
            waits = list(si.on_wait) if (si and si.on_wait) else []
            if len(waits) > max_waits:
                extra, keep = waits[:-max_waits], waits[-max_waits:]
                for i in range(0, len(extra), max_waits):
                    carrier = mybir.InstDrain(
                        name=f"{ins.name}_wsplit{i}", ins=[], outs=[],
                        bass_is_fusable=False)
                    carrier.engine = ins.engine
                    carrier.sync_info = mybir.SyncInfo(
                        on_wait=extra[i:i + max_waits], on_update=[])
                    out.append(carrier)
                    n += 1
                si.on_wait = keep
            out.append(ins)
        blk.instructions = out
    return n


# ---- packed single-input blob layout (bf16 rows of 2048 = 4KB each) ----
# Per-call cost of the PJRT exec path is dominated by a ~1.2ms per-input-
# tensor overhead plus ~0.06ms/MB of staged bytes, so every per-core
# tensor is packed into ONE bf16 blob, and all graph-shared data (A +
# weights, identical on every core) is sharded 8 ways and rebuilt
# on-device with an AllGather. Gathered region G row offsets:
G_AT = 0              # at fp8e4m3 [2048, 2048] -> 1024 bf16 rows
G_NODE = 1024         # node_emb bf16 flat (2048x16 -> 16 rows)
G_NODET = 1040        # node_emb.T f32 [16, 2048] -> 32 rows
G_WGE = 1072          # wge f32 [98, 512] -> 49 rows (r|u packed per group)
G_WCE = G_WGE + 49    # wce f32 [64, 384] -> 24 rows
G_WGD = G_WCE + 24    # wgd f32 [66, 512] -> 33 rows
G_WCD = G_WGD + 33    # wcd f32 [64, 384] -> 24 rows
G_WFE = G_WCD + 24    # wfe rows @ 0/256, wproj @ 512 (f32) -> 1 row
G_BIAS = G_WFE + 1    # bias [64,8] @ 0, gate r|u bias pairs [128,2] @ 512
G_TOT = 1208          # padded to 8 * 151
SHARD = G_TOT // NCORES
R_XS = SHARD          # xs f32 [12, 2048] -> 24 rows (per-core unique)
R_TOT = R_XS + 24


def _build():
    nc = bass.Bass(num_devices=NCORES)

    blob_d = nc.dram_tensor("blob", [R_TOT, N], BF16, kind="ExternalInput")
    ats_d = nc.dram_tensor("ats", [SHARD, N], BF16, kind="Internal")
    atg_d = nc.dram_tensor("atg", [G_TOT, N], BF16, kind="Internal",
                           addr_space="Shared")
    out_d = nc.dram_tensor("out", [HOR, N], F32R, kind="ExternalOutput")

    with tile.TileContext(nc) as tc:
        with tc.tile_pool(name="const", bufs=1) as cpool, \
             tc.tile_pool(name="state", bufs=1) as spool, \
             tc.tile_pool(name="psum", bufs=1, space="PSUM") as ppool:

            at_sb = cpool.tile([P, NT * N], FP8, name="at_sb")
            z_nat = spool.tile([P, NT * ZE], BF16, name="z_nat")
            h1_nat = spool.tile([P, NT * ZE], BF16, name="h1_nat")
            zd_nat = spool.tile([P, NT * ZD], BF16, name="zd_nat")
            rh_nat = spool.tile([P, NT * H], BF16, name="rh_nat")
            ch_nat = spool.tile([P, NT * H], BF16, name="ch_nat")
            zxT = spool.tile([CX, N], F32R, name="zxT")  # dec: row0 = yT
            hT = spool.tile([H, N], F32R, name="hT")
            h1T = spool.tile([ZE, N], F32R, name="h1T")
            h2T = spool.tile([ZE, N], F32R, name="h2T")
            c1T = spool.tile([H, N], F32R, name="c1T")
            c2T = spool.tile([H, N], F32R, name="c2T")
            rT = spool.tile([H, N], F32R, name="rT")
            uT = spool.tile([H, N], F32R, name="uT")
            rhT = spool.tile([H, N], F32R, name="rhT")
            cT = spool.tile([H, N], F32R, name="cT")
            sc1 = spool.tile([H, N], F32R, name="sc1")
            xcur = spool.tile([2, N], F32, name="xcur")
            wfe_sb = cpool.tile([2, 256], F32, name="wfe_sb")
            wge_sb = cpool.tile([98, 512], F32R, name="wge_sb")
            wce_sb = cpool.tile([64, 384], F32R, name="wce_sb")
            wgd_sb = cpool.tile([66, 512], F32R, name="wgd_sb")
            wcd_sb = cpool.tile([64, 384], F32R, name="wcd_sb")
            wpj_sb = cpool.tile([64, 1], F32R, name="wpj_sb")
            bias_sb = cpool.tile([64, 8], F32, name="bias_sb")
            ident = cpool.tile([P, P], F32, name="ident")

            make_identity(nc, ident[:, :])

            # ---------- prologue: gather shared data, unpack ----------
            # Graph-shared data is sharded 8-ways across cores; allgather
            # rebuilds the full region on-device so each call only stages
            # ~600KB/core through the PJRT input path.
            nc.sync.dma_start(ats_d[:, :], blob_d[0:SHARD, :])
            nc.gpsimd.collective_compute(
                "AllGather", mybir.AluOpType.bypass,
                replica_groups=[list(range(NCORES))],
                ins=[ats_d[:, :]],
                outs=[atg_d[:, :]])
            for k in range(NT):
                nc.sync.dma_start(at_sb[:, k * N:(k + 1) * N],
                                  atg_d[G_AT + k * 64:G_AT + (k + 1) * 64, :]
                                  .bitcast(FP8))
            for j in range(NT):
                # one 4KB row holds 128 nodes x 16 emb, row-major
                nc.sync.dma_start(z_nat[:, j * ZE + 17:j * ZE + 33],
                                  atg_d[G_NODE + j:G_NODE + j + 1, :])
            nc.sync.dma_start(zxT[17:33, :],
                              atg_d[G_NODET:G_NODET + 32, :].bitcast(F32R))
            nc.sync.dma_start(wfe_sb[:, :],
                              atg_d[G_WFE:G_WFE + 1, 0:1024].bitcast(F32))
            nc.sync.dma_start(wpj_sb[:, :],
                              atg_d[G_WFE:G_WFE + 1, 1024:1152].bitcast(F32R))
            nc.sync.dma_start(wge_sb[:, :],
                              atg_d[G_WGE:G_WGE + 49, :].bitcast(F32R))
            nc.sync.dma_start(wce_sb[:, :],
                              atg_d[G_WCE:G_WCE + 24, :].bitcast(F32R))
            nc.sync.dma_start(wgd_sb[:, :],
                              atg_d[G_WGD:G_WGD + 33, :].bitcast(F32R))
            nc.sync.dma_start(wcd_sb[:, :],
                              atg_d[G_WCD:G_WCD + 24, :].bitcast(F32R))
            nc.sync.dma_start(bias_sb[:, :],
                              atg_d[G_BIAS:G_BIAS + 1, 0:1024].bitcast(F32))
            # row1 stays 1.0 (bias/ones row); row0 is overwritten by the
            # per-step x DMA. memset both rows: partition base must be 0.
            nc.vector.memset(xcur[0:2, :], 1.0)
            nc.vector.memset(hT[:, :].bitcast(F32), 0.0)
            for j in range(NT):
                nc.vector.memset(z_nat[:, j * ZE + 33:(j + 1) * ZE], 0.0)

            # ---------- helpers ----------
            def copy_cast(dst, src, alt):
                if alt % 2 == 0:
                    nc.vector.tensor_copy(dst, src)
                else:
                    nc.scalar.copy(dst, src)

            def hop_pass(lhs_nat, stride, m, outT):
                # outT[0:m, :] = (A @ z).T given z natural tiles in lhs_nat
                for c in range(NCH):
                    hp = ppool.tile([m, FREE], F32, name="hp", tag="hp",
                                    bufs=2)
                    for k in range(NT):
                        nc.tensor.matmul(
                            hp[:, :],
                            lhs_nat[:, k * stride:k * stride + m],
                            at_sb[:, k * N + c * FREE:k * N + (c + 1) * FREE],
                            start=(k == 0), stop=(k == NT - 1))
                    # split the copy across both engines: this copy gates
                    # the transposes feeding the next chained hop pass.
                    half = FREE // 2
                    nc.vector.tensor_copy(
                        outT[0:m, c * FREE:c * FREE + half], hp[:, 0:half])
                    nc.scalar.copy(
                        outT[0:m, c * FREE + half:(c + 1) * FREE],
                        hp[:, half:FREE])

            def to_nat(srcT, m, dst, stride, off, cast_start=0):
                # dst[:, j*stride+off : +m] (bf16) = srcT[0:m, j*128:+128].T
                for j in range(NT):
                    tp = ppool.tile([P, m], F32, name="tp", tag="tp", bufs=4)
                    nc.tensor.transpose(
                        tp[:, :],
                        srcT[0:m, j * P:(j + 1) * P].bitcast(F32),
                        ident[0:m, 0:m])
                    copy_cast(dst[:, j * stride + off:j * stride + off + m],
                              tp[:, :], j + cast_start)

            def dense_gate(groups, w_sb, cr, cu):
                # [rT|uT] = sigmoid(sum_g w_g.T @ rhs_g + bias), one 128-wide
                # matmul accumulation per chunk, split into r/u activations
                for c in range(NCH):
                    dp = ppool.tile([P, FREE], F32, name="dpg", tag="dp",
                                    bufs=2)
                    ng = len(groups)
                    for gi, (off, kr, rhs) in enumerate(groups):
                        nc.tensor.matmul(
                            dp[:, :],
                            w_sb[0:kr, off:off + P],
                            rhs[0:kr, c * FREE:(c + 1) * FREE],
                            start=(gi == 0), stop=(gi == ng - 1))
                    nc.scalar.activation(rT[0:H, c * FREE:(c + 1) * FREE],
                                         dp[0:H, :], AF.Sigmoid,
                                         bias=bias_sb[0:H, cr:cr + 1])
                    nc.scalar.activation(uT[0:H, c * FREE:(c + 1) * FREE],
                                         dp[H:P, :], AF.Sigmoid,
                                         bias=bias_sb[0:H, cu:cu + 1])

            def dense(groups, w_sb, outT, func, bias_ap, m=H):
                # outT[0:m, :] = func(sum_g w_g.T @ rhs_g + bias)
                for c in range(NCH):
                    dp = ppool.tile([m, FREE], F32, name="dp", tag="dp",
                                    bufs=2)
                    ng = len(groups)
                    for gi, (off, kr, rhs) in enumerate(groups):
                        nc.tensor.matmul(
                            dp[:, :],
                            w_sb[0:kr, off:off + m],
                            rhs[0:kr, c * FREE:(c + 1) * FREE],
                            start=(gi == 0), stop=(gi == ng - 1))
                    nc.scalar.activation(outT[0:m, c * FREE:(c + 1) * FREE],
                                         dp[:, :], func, bias=bias_ap)

            def update_h(nat_dst, stride, off):
                # h' = c + u*(h-c); write h' (f32) and its natural bf16 tiles
                nc.vector.tensor_sub(sc1[:, :], hT[:, :], cT[:, :])
                nc.vector.tensor_mul(sc1[:, :], sc1[:, :], uT[:, :])
                nc.vector.tensor_add(hT[:, :], sc1[:, :], cT[:, :])
                to_nat(hT, H, nat_dst, stride, off, 1)

            # ---------- encoder ----------
            for t in range(S):
                nc.sync.dma_start(xcur[0:1, :],
                                  blob_d[R_XS + 2 * t:R_XS + 2 * t + 2, :]
                                  .bitcast(F32))
                # featx channel-major rows (zxT[0:17]) and natural cols
                for c in range(NCH):
                    fx = ppool.tile([17, FREE], F32, name="fx", tag="dp",
                                    bufs=2)
                    nc.tensor.matmul(fx[:, :], wfe_sb[:, 0:17],
                                     xcur[:, c * FREE:(c + 1) * FREE],
                                     start=True, stop=True)
                    nc.scalar.copy(zxT[0:17, c * FREE:(c + 1) * FREE], fx[:, :])
                for j in range(NT):
                    fn = ppool.tile([P, 17], F32, name="fn", tag="tp", bufs=4)
                    nc.tensor.matmul(fn[:, :], xcur[:, j * P:(j + 1) * P],
                                     wfe_sb[:, 0:17], start=True, stop=True)
                    copy_cast(z_nat[:, j * ZE:j * ZE + 17], fn[:, :], j)

                hop_pass(z_nat, ZE, ZE, h1T)
                to_nat(h1T, ZE, h1_nat, ZE, 0)
                hop_pass(h1_nat, ZE, ZE, h2T)
                dense_gate([(0, CX, zxT), (128, H, hT), (256, ZE, h1T),
                            (384, ZE, h2T)], wge_sb, 0, 1)
                nc.vector.tensor_mul(rhT[:, :], rT[:, :], hT[:, :])
                to_nat(rhT, H, rh_nat, H, 0)
                hop_pass(rh_nat, H, H, c1T)
                to_nat(c1T, H, ch_nat, H, 0, 1)
                hop_pass(ch_nat, H, H, c2T)
                dense([(0, CX, zxT), (64, H, rhT), (128, CX, h1T),
                       (192, H, c1T), (256, CX, h2T), (320, H, c2T)],
                      wce_sb, cT, AF.Tanh, bias_sb[0:H, 2:3])
                if t < S - 1:
                    update_h(z_nat, ZE, 33)
                else:
                    update_h(zd_nat, ZD, 1)

            # ---------- decoder ----------
            nc.vector.memset(zxT[0:1, :].bitcast(F32), 0.0)  # GO symbol y=0
            for j in range(NT):
                nc.vector.memset(zd_nat[:, j * ZD:j * ZD + 1], 0.0)

            for u in range(HOR):
                hop_pass(zd_nat, ZD, ZD, h1T)
                to_nat(h1T, ZD, h1_nat, ZD, 0)
                hop_pass(h1_nat, ZD, ZD, h2T)
                dense_gate([(0, 1, zxT), (128, H, hT), (256, ZD, h1T),
                            (384, ZD, h2T)], wgd_sb, 3, 4)
                nc.vector.tensor_mul(rhT[:, :], rT[:, :], hT[:, :])
                to_nat(rhT, H, rh_nat, H, 0)
                hop_pass(rh_nat, H, H, c1T)
                to_nat(c1T, H, ch_nat, H, 0, 1)
                hop_pass(ch_nat, H, H, c2T)
                dense([(0, 1, zxT), (64, H, rhT), (128, 1, h1T),
                       (192, H, c1T), (256, 1, h2T), (320, H, c2T)],
                      wcd_sb, cT, AF.Tanh, bias_sb[0:H, 5:6])
                update_h(zd_nat, ZD, 1)
                # y = h' @ Wproj + b  -> zxT row 0 (channel-major y)
                for c in range(NCH):
                    yp = ppool.tile([1, FREE], F32, name="yp", tag="dp",
                                    bufs=2)
                    nc.tensor.matmul(yp[:, :], wpj_sb[:, :],
                                     hT[:, c * FREE:(c + 1) * FREE],
                                     start=True, stop=True)
                    nc.scalar.activation(zxT[0:1, c * FREE:(c + 1) * FREE],
                                         yp[:, :], AF.Identity,
                                         bias=bias_sb[0:1, 6:7])
                nc.sync.dma_start(out_d[u:u + 1, :], zxT[0:1, :])
                if u < HOR - 1:
                    for j in range(NT):
                        ty = ppool.tile([P, 1], F32, name="ty", tag="tp",
                                        bufs=4)
                        nc.tensor.transpose(
                            ty[:, :],
                            zxT[0:1, j * P:(j + 1) * P].bitcast(F32),
                            ident[0:1, 0:1])
                        copy_cast(zd_nat[:, j * ZD:j * ZD + 1], ty[:, :], j)

    _split_multiwait(nc)
    return nc


# ---------------- host-side preprocessing ----------------

def _softplus(x):
    return np.log1p(np.exp(-np.abs(x))) + np.maximum(x, 0.0)


def _host_prep(inp):
    """Edge-weight MLP + row-normalization + dense A^T build + weight
    packing/permutation. Pure per-graph preprocessing (no time loop)."""
    f = np.float32
    row, col = np.asarray(inp["sparse_idx"])
    loc = np.asarray(inp["loc"], f)
    delta = loc[col] - loc[row]
    h1 = np.tanh(delta @ np.asarray(inp["Wk0"], f) + np.asarray(inp["bk0"], f))
    h2 = np.tanh(h1 @ np.asarray(inp["Wk1"], f) + np.asarray(inp["bk1"], f))
    ker = _softplus((h2 @ np.asarray(inp["Wk2"], f)
                     + np.asarray(inp["bk2"], f))[:, 0])
    geo = np.asarray(inp["geodesic"], f)
    w = ker * np.asarray(inp["angle_ratio"], f) * np.exp(-geo * geo)
    denom = np.zeros(N, f)
    np.add.at(denom, row, w)
    w = (w / (denom[row] + np.float32(1e-8))).astype(f)
    at = np.zeros((N, N), f)
    np.add.at(at, (col, row), w)          # at[m, n] = A[n, m]

    # channel permutation: reference z order [feat16|node16|x1|h64]
    # -> ours [feat16|x1|node16|h64]
    px = np.concatenate([np.arange(16), [32], np.arange(16, 32)])
    ph = np.arange(33, 97)

    wg = np.asarray(inp["Wg_e"], f)       # (291, 128)
    wc = np.asarray(inp["Wc_e"], f)       # (291, 64)
    gblocks = [px, ph, np.concatenate([97 + px, 97 + ph]),
               np.concatenate([194 + px, 194 + ph])]
    wge = np.zeros((97, 512), f)
    for i, b in enumerate(gblocks):
        wge[:len(b), i * 128:i * 128 + 128] = wg[b, :]
    cblocks = [px, ph, 97 + px, 97 + ph, 194 + px, 194 + ph]
    wce = np.zeros((64, 384), f)
    for i, b in enumerate(cblocks):
        wce[:len(b), i * 64:i * 64 + 64] = wc[b]

    wgd_r = np.asarray(inp["Wg_d"], f)    # (195, 128)
    wcd_r = np.asarray(inp["Wc_d"], f)    # (195, 64)
    dgblocks = [np.arange(0, 1), np.arange(1, 65), np.arange(65, 130),
                np.arange(130, 195)]
    wgd = np.zeros((65, 512), f)
    for i, b in enumerate(dgblocks):
        wgd[:len(b), i * 128:i * 128 + 128] = wgd_r[b, :]
    dcblocks = [np.arange(0, 1), np.arange(1, 65), np.arange(65, 66),
                np.arange(66, 130), np.arange(130, 131), np.arange(131, 195)]
    wcd = np.zeros((64, 384), f)
    for i, b in enumerate(dcblocks):
        wcd[:len(b), i * 64:i * 64 + 64] = wcd_r[b]

    wfe = np.zeros((2, 17), f)
    wfe[0, 0:16] = np.asarray(inp["W_fe"], f)[0]
    wfe[0, 16] = 1.0
    wfe[1, 0:16] = np.asarray(inp["b_fe"], f)
    bias = np.zeros((64, 8), f)
    bias[:, 0] = np.asarray(inp["bg_e"], f)[0:64]
    bias[:, 1] = np.asarray(inp["bg_e"], f)[64:128]
    bias[:, 2] = np.asarray(inp["bc_e"], f)
    bias[:, 3] = np.asarray(inp["bg_d"], f)[0:64]
    bias[:, 4] = np.asarray(inp["bg_d"], f)[64:128]
    bias[:, 5] = np.asarray(inp["bc_d"], f)
    bias[0, 6] = np.asarray(inp["b_proj"], f)[0]

    node = np.asarray(inp["node_emb"], f)
    bf = ml_dtypes.bfloat16

    def put_f32(blob, r0, nrows, arr):
        v = blob[r0:r0 + nrows].view(np.float32)
        fl = v.reshape(-1)
        a = arr.reshape(-1)
        fl[:a.size] = a

    shared = np.zeros((G_TOT, N), bf)
    shared[G_AT:G_AT + 1024].view(ml_dtypes.float8_e4m3)[:] = (
        at.astype(ml_dtypes.float8_e4m3).reshape(1024, 2 * N))
    shared[G_NODE:G_NODE + EMB] = node.astype(bf).reshape(EMB, N)
    put_f32(shared, G_NODET, 32, np.ascontiguousarray(node.T))
    wge_p = np.zeros((98, 512), f)
    wge_p[:97] = wge
    put_f32(shared, G_WGE, 49, wge_p)
    put_f32(shared, G_WCE, 24, wce)
    wgd_p = np.zeros((66, 512), f)
    wgd_p[:65] = wgd
    put_f32(shared, G_WGD, 33, wgd_p)
    put_f32(shared, G_WCD, 24, wcd)
    frow = np.zeros(1024, f)
    frow[0:17] = wfe[0]
    frow[256:273] = wfe[1]
    frow[512:576] = np.asarray(inp["W_proj"], f)[:, 0]
    put_f32(shared, G_WFE, 1, frow)
    put_f32(shared, G_BIAS, 1, bias)

    xs = np.asarray(inp["inputs"], f)     # (S, B, N, 1)
    in_maps = []
    for b in range(NCORES):
        blob = np.zeros((R_TOT, N), bf)
        blob[0:SHARD] = shared[b * SHARD:(b + 1) * SHARD]
        put_f32(blob, R_XS, 24, np.ascontiguousarray(xs[:, b, :, 0]))
        in_maps.append({"blob": blob})
    return in_maps


_NC_CACHE = []


def kernel(**inputs):
    if not _NC_CACHE:
        _NC_CACHE.append(_build())
    nc = _NC_CACHE[0]
    in_maps = _host_prep(inputs)
    res = run_bass_kernel_spmd(nc, in_maps, core_ids=list(range(NCORES)))
    out = np.stack([res.results[b]["out"] for b in range(NCORES)], axis=1)
    return np.ascontiguousarray(out[..., None].astype(np.float32))



# revision 17
# speedup vs baseline: 2.9300x; 2.9300x over previous
"""Trainium2 Bass kernel for nn_CLCRNModel (CLCRN encoder-decoder GNN).

Strategy: data-parallel over batch (8 batch elements -> 8 NeuronCores).
The sparse 25-neighbor graph conv is cast as a dense matmul against the
(2048, 2048) row-normalized adjacency A kept resident in SBUF (bf16).
Per GRU cell: 4 chained A-passes on the PE (gate 2-hop + candidate 2-hop)
plus small dense weight matmuls in fp32r. Activations flow in two layouts:
channel-major (C, N) for dense-W rhs / elementwise, and natural
node-major (128, C) tiles (bf16) as matmul stationary operands, produced
by PE transposes.
"""
import os
import sys

for _p in ("/root/.axon_site/_ro/trn_rl_repo", "/opt/trn_rl_repo"):
    if os.path.isdir(_p) and _p not in sys.path:
        sys.path.append(_p)

import numpy as np
import ml_dtypes

import concourse.bass as bass
import concourse.mybir as mybir
import concourse.tile as tile
from concourse.bass_utils import run_bass_kernel_spmd
from concourse.masks import make_identity

P = 128
N = 2048
NT = 16            # node tiles
S = 12             # encoder steps
HOR = 12           # decoder steps
EMB = 16
H = 64             # GRU units
CX = 33            # encoder x-part channels [feat16 | x1 | node16]
ZE = 97            # encoder z channels (CX + H)
ZD = 65            # decoder z channels (1 + H)
FREE = 512
NCH = N // FREE    # 4 free chunks
NCORES = 8

F32 = mybir.dt.float32
F32R = mybir.dt.float32r
BF16 = mybir.dt.bfloat16
FP8 = mybir.dt.float8e4
AF = mybir.ActivationFunctionType


def _r(ap):
    return ap.bitcast(F32R)


def _split_multiwait(nc, max_waits=1):
    """This container's walrus rejects >1 sem-wait on CTRL-class
    instructions (the Tile exit drain carries one wait per live sem).
    Split excess waits onto preceding same-engine carrier drains."""
    fn = nc.m.functions[0]
    n = 0
    for blk in fn.blocks:
        out = []
        for ins in blk.instructions:
            si = ins.sync_info
            waits = list(si.on_wait) if (si and si.on_wait) else []
            if len(waits) > max_waits:
                extra, keep = waits[:-max_waits], waits[-max_waits:]
                for i in range(0, len(extra), max_waits):
                    carrier = mybir.InstDrain(
                        name=f"{ins.name}_wsplit{i}", ins=[], outs=[],
                        bass_is_fusable=False)
                    carrier.engine = ins.engine
                    carrier.sync_info = mybir.SyncInfo(
                        on_wait=extra[i:i + max_waits], on_update=[])
                    out.append(carrier)
                    n += 1
                si.on_wait = keep
            out.append(ins)
        blk.instructions = out
    return n


# ---- packed single-input blob layout (bf16 rows of 2048 = 4KB each) ----
# Per-call cost of the PJRT exec path is dominated by a ~1.2ms per-input-
# tensor overhead plus ~0.06ms/MB of staged bytes, so every per-core
# tensor is packed into ONE bf16 blob, and all graph-shared data (A +
# weights, identical on every core) is sharded 8 ways and rebuilt
# on-device with an AllGather. Gathered region G row offsets:
G_AT = 0              # at fp8e4m3 [2048, 2048] -> 1024 bf16 rows
G_NODE = 1024         # node_emb bf16 flat (2048x16 -> 16 rows)
G_NODET = 1040        # node_emb.T f32 [16, 2048] -> 32 rows
G_WGE = 1072          # wge f32 [98, 512] -> 49 rows (r|u packed per group)
G_WCE = G_WGE + 49    # wce f32 [64, 384] -> 24 rows
G_WGD = G_WCE + 24    # wgd f32 [66, 512] -> 33 rows
G_WCD = G_WGD + 33    # wcd f32 [64, 384] -> 24 rows
G_WFE = G_WCD + 24    # wfe rows @ 0/256, wproj @ 512 (f32) -> 1 row
G_BIAS = G_WFE + 1    # bias [64,8] @ 0, gate r|u bias pairs [128,2] @ 512
G_TOT = 1208          # padded to 8 * 151
SHARD = G_TOT // NCORES
# USE_CC: shard the shared region across cores + on-device AllGather
# (7/8 less staging, but adds a per-exec cross-core sync point).
USE_CC = False
R_XS = SHARD if USE_CC else G_TOT
R_TOT = R_XS + 24


def _build():
    nc = bass.Bass(num_devices=NCORES)

    blob_d = nc.dram_tensor("blob", [R_TOT, N], BF16, kind="ExternalInput")
    if USE_CC:
        ats_d = nc.dram_tensor("ats", [SHARD, N], BF16, kind="Internal")
        atg_d = nc.dram_tensor("atg", [G_TOT, N], BF16, kind="Internal",
                               addr_space="Shared")
    else:
        atg_d = blob_d
    out_d = nc.dram_tensor("out", [HOR, N], F32R, kind="ExternalOutput")

    with tile.TileContext(nc) as tc:
        with tc.tile_pool(name="const", bufs=1) as cpool, \
             tc.tile_pool(name="state", bufs=1) as spool, \
             tc.tile_pool(name="psum", bufs=1, space="PSUM") as ppool:

            at_sb = cpool.tile([P, NT * N], FP8, name="at_sb")
            z_nat = spool.tile([P, NT * ZE], BF16, name="z_nat")
            h1_nat = spool.tile([P, NT * ZE], BF16, name="h1_nat")
            zd_nat = spool.tile([P, NT * ZD], BF16, name="zd_nat")
            rh_nat = spool.tile([P, NT * H], BF16, name="rh_nat")
            ch_nat = spool.tile([P, NT * H], BF16, name="ch_nat")
            zxT = spool.tile([CX, N], F32R, name="zxT")  # dec: row0 = yT
            hT = spool.tile([H, N], F32R, name="hT")
            h1T = spool.tile([ZE, N], F32R, name="h1T")
            h2T = spool.tile([ZE, N], F32R, name="h2T")
            c1T = spool.tile([H, N], F32R, name="c1T")
            c2T = spool.tile([H, N], F32R, name="c2T")
            rT = spool.tile([H, N], F32R, name="rT")
            uT = spool.tile([H, N], F32R, name="uT")
            rhT = spool.tile([H, N], F32R, name="rhT")
            cT = spool.tile([H, N], F32R, name="cT")
            sc1 = spool.tile([H, N], F32R, name="sc1")
            xcur = spool.tile([2, N], F32, name="xcur")
            wfe_sb = cpool.tile([2, 256], F32, name="wfe_sb")
            wge_sb = cpool.tile([98, 512], F32R, name="wge_sb")
            wce_sb = cpool.tile([64, 384], F32R, name="wce_sb")
            wgd_sb = cpool.tile([66, 512], F32R, name="wgd_sb")
            wcd_sb = cpool.tile([64, 384], F32R, name="wcd_sb")
            wpj_sb = cpool.tile([64, 1], F32R, name="wpj_sb")
            bias_sb = cpool.tile([64, 8], F32, name="bias_sb")
            ident = cpool.tile([P, P], F32, name="ident")

            make_identity(nc, ident[:, :])

            # ---------- prologue: gather shared data, unpack ----------
            # Graph-shared data is sharded 8-ways across cores; allgather
            # rebuilds the full region on-device so each call only stages
            # ~600KB/core through the PJRT input path.
            if USE_CC:
                nc.sync.dma_start(ats_d[:, :], blob_d[0:SHARD, :])
                nc.gpsimd.collective_compute(
                    "AllGather", mybir.AluOpType.bypass,
                    replica_groups=[list(range(NCORES))],
                    ins=[ats_d[:, :]],
                    outs=[atg_d[:, :]])
            for k in range(NT):
                nc.sync.dma_start(at_sb[:, k * N:(k + 1) * N],
                                  atg_d[G_AT + k * 64:G_AT + (k + 1) * 64, :]
                                  .bitcast(FP8))
            for j in range(NT):
                # one 4KB row holds 128 nodes x 16 emb, row-major
                nc.sync.dma_start(z_nat[:, j * ZE + 17:j * ZE + 33],
                                  atg_d[G_NODE + j:G_NODE + j + 1, :])
            nc.sync.dma_start(zxT[17:33, :],
                              atg_d[G_NODET:G_NODET + 32, :].bitcast(F32R))
            nc.sync.dma_start(wfe_sb[:, :],
                              atg_d[G_WFE:G_WFE + 1, 0:1024].bitcast(F32))
            nc.sync.dma_start(wpj_sb[:, :],
                              atg_d[G_WFE:G_WFE + 1, 1024:1152].bitcast(F32R))
            nc.sync.dma_start(wge_sb[:, :],
                              atg_d[G_WGE:G_WGE + 49, :].bitcast(F32R))
            nc.sync.dma_start(wce_sb[:, :],
                              atg_d[G_WCE:G_WCE + 24, :].bitcast(F32R))
            nc.sync.dma_start(wgd_sb[:, :],
                              atg_d[G_WGD:G_WGD + 33, :].bitcast(F32R))
            nc.sync.dma_start(wcd_sb[:, :],
                              atg_d[G_WCD:G_WCD + 24, :].bitcast(F32R))
            nc.sync.dma_start(bias_sb[:, :],
                              atg_d[G_BIAS:G_BIAS + 1, 0:1024].bitcast(F32))
            # row1 stays 1.0 (bias/ones row); row0 is overwritten by the
            # per-step x DMA. memset both rows: partition base must be 0.
            nc.vector.memset(xcur[0:2, :], 1.0)
            nc.vector.memset(hT[:, :].bitcast(F32), 0.0)
            for j in range(NT):
                nc.vector.memset(z_nat[:, j * ZE + 33:(j + 1) * ZE], 0.0)

            # ---------- helpers ----------
            def copy_cast(dst, src, alt):
                if alt % 2 == 0:
                    nc.vector.tensor_copy(dst, src)
                else:
                    nc.scalar.copy(dst, src)

            def hop_pass(lhs_nat, stride, m, outT):
                # outT[0:m, :] = (A @ z).T given z natural tiles in lhs_nat
                for c in range(NCH):
                    hp = ppool.tile([m, FREE], F32, name="hp", tag="hp",
                                    bufs=2)
                    for k in range(NT):
                        nc.tensor.matmul(
                            hp[:, :],
                            lhs_nat[:, k * stride:k * stride + m],
                            at_sb[:, k * N + c * FREE:k * N + (c + 1) * FREE],
                            start=(k == 0), stop=(k == NT - 1))
                    # split the copy across both engines: this copy gates
                    # the transposes feeding the next chained hop pass.
                    half = FREE // 2
                    nc.vector.tensor_copy(
                        outT[0:m, c * FREE:c * FREE + half], hp[:, 0:half])
                    nc.scalar.copy(
                        outT[0:m, c * FREE + half:(c + 1) * FREE],
                        hp[:, half:FREE])

            def to_nat(srcT, m, dst, stride, off, cast_start=0):
                # dst[:, j*stride+off : +m] (bf16) = srcT[0:m, j*128:+128].T
                for j in range(NT):
                    tp = ppool.tile([P, m], F32, name="tp", tag="tp", bufs=4)
                    nc.tensor.transpose(
                        tp[:, :],
                        srcT[0:m, j * P:(j + 1) * P].bitcast(F32),
                        ident[0:m, 0:m])
                    copy_cast(dst[:, j * stride + off:j * stride + off + m],
                              tp[:, :], j + cast_start)

            def dense_gate(groups, w_sb, cr, cu):
                # [rT|uT] = sigmoid(sum_g w_g.T @ rhs_g + bias), one 128-wide
                # matmul accumulation per chunk, split into r/u activations
                for c in range(NCH):
                    dp = ppool.tile([P, FREE], F32, name="dpg", tag="dp",
                                    bufs=2)
                    ng = len(groups)
                    for gi, (off, kr, rhs) in enumerate(groups):
                        nc.tensor.matmul(
                            dp[:, :],
                            w_sb[0:kr, off:off + P],
                            rhs[0:kr, c * FREE:(c + 1) * FREE],
                            start=(gi == 0), stop=(gi == ng - 1))
                    nc.scalar.activation(rT[0:H, c * FREE:(c + 1) * FREE],
                                         dp[0:H, :], AF.Sigmoid,
                                         bias=bias_sb[0:H, cr:cr + 1])
                    nc.scalar.activation(uT[0:H, c * FREE:(c + 1) * FREE],
                                         dp[H:P, :], AF.Sigmoid,
                                         bias=bias_sb[0:H, cu:cu + 1])

            def dense(groups, w_sb, outT, func, bias_ap, m=H):
                # outT[0:m, :] = func(sum_g w_g.T @ rhs_g + bias)
                for c in range(NCH):
                    dp = ppool.tile([m, FREE], F32, name="dp", tag="dp",
                                    bufs=2)
                    ng = len(groups)
                    for gi, (off, kr, rhs) in enumerate(groups):
                        nc.tensor.matmul(
                            dp[:, :],
                            w_sb[0:kr, off:off + m],
                            rhs[0:kr, c * FREE:(c + 1) * FREE],
                            start=(gi == 0), stop=(gi == ng - 1))
                    nc.scalar.activation(outT[0:m, c * FREE:(c + 1) * FREE],
                                         dp[:, :], func, bias=bias_ap)

            def update_h(nat_dst, stride, off):
                # h' = c + u*(h-c); write h' (f32) and its natural bf16 tiles
                nc.vector.tensor_sub(sc1[:, :], hT[:, :], cT[:, :])
                nc.vector.tensor_mul(sc1[:, :], sc1[:, :], uT[:, :])
                nc.vector.tensor_add(hT[:, :], sc1[:, :], cT[:, :])
                to_nat(hT, H, nat_dst, stride, off, 1)

            # ---------- encoder ----------
            for t in range(S):
                nc.sync.dma_start(xcur[0:1, :],
                                  blob_d[R_XS + 2 * t:R_XS + 2 * t + 2, :]
                                  .bitcast(F32))
                # featx channel-major rows (zxT[0:17]) and natural cols
                for c in range(NCH):
                    fx = ppool.tile([17, FREE], F32, name="fx", tag="dp",
                                    bufs=2)
                    nc.tensor.matmul(fx[:, :], wfe_sb[:, 0:17],
                                     xcur[:, c * FREE:(c + 1) * FREE],
                                     start=True, stop=True)
                    nc.scalar.copy(zxT[0:17, c * FREE:(c + 1) * FREE], fx[:, :])
                for j in range(NT):
                    fn = ppool.tile([P, 17], F32, name="fn", tag="tp", bufs=4)
                    nc.tensor.matmul(fn[:, :], xcur[:, j * P:(j + 1) * P],
                                     wfe_sb[:, 0:17], start=True, stop=True)
                    copy_cast(z_nat[:, j * ZE:j * ZE + 17], fn[:, :], j)

                hop_pass(z_nat, ZE, ZE, h1T)
                to_nat(h1T, ZE, h1_nat, ZE, 0)
                hop_pass(h1_nat, ZE, ZE, h2T)
                dense_gate([(0, CX, zxT), (128, H, hT), (256, ZE, h1T),
                            (384, ZE, h2T)], wge_sb, 0, 1)
                nc.vector.tensor_mul(rhT[:, :], rT[:, :], hT[:, :])
                to_nat(rhT, H, rh_nat, H, 0)
                hop_pass(rh_nat, H, H, c1T)
                to_nat(c1T, H, ch_nat, H, 0, 1)
                hop_pass(ch_nat, H, H, c2T)
                dense([(0, CX, zxT), (64, H, rhT), (128, CX, h1T),
                       (192, H, c1T), (256, CX, h2T), (320, H, c2T)],
                      wce_sb, cT, AF.Tanh, bias_sb[0:H, 2:3])
                if t < S - 1:
                    update_h(z_nat, ZE, 33)
                else:
                    update_h(zd_nat, ZD, 1)

            # ---------- decoder ----------
            nc.vector.memset(zxT[0:1, :].bitcast(F32), 0.0)  # GO symbol y=0
            for j in range(NT):
                nc.vector.memset(zd_nat[:, j * ZD:j * ZD + 1], 0.0)

            for u in range(HOR):
                hop_pass(zd_nat, ZD, ZD, h1T)
                to_nat(h1T, ZD, h1_nat, ZD, 0)
                hop_pass(h1_nat, ZD, ZD, h2T)
                dense_gate([(0, 1, zxT), (128, H, hT), (256, ZD, h1T),
                            (384, ZD, h2T)], wgd_sb, 3, 4)
                nc.vector.tensor_mul(rhT[:, :], rT[:, :], hT[:, :])
                to_nat(rhT, H, rh_nat, H, 0)
                hop_pass(rh_nat, H, H, c1T)
                to_nat(c1T, H, ch_nat, H, 0, 1)
                hop_pass(ch_nat, H, H, c2T)
                dense([(0, 1, zxT), (64, H, rhT), (128, 1, h1T),
                       (192, H, c1T), (256, 1, h2T), (320, H, c2T)],
                      wcd_sb, cT, AF.Tanh, bias_sb[0:H, 5:6])
                update_h(zd_nat, ZD, 1)
                # y = h' @ Wproj + b  -> zxT row 0 (channel-major y)
                for c in range(NCH):
                    yp = ppool.tile([1, FREE], F32, name="yp", tag="dp",
                                    bufs=2)
                    nc.tensor.matmul(yp[:, :], wpj_sb[:, :],
                                     hT[:, c * FREE:(c + 1) * FREE],
                                     start=True, stop=True)
                    nc.scalar.activation(zxT[0:1, c * FREE:(c + 1) * FREE],
                                         yp[:, :], AF.Identity,
                                         bias=bias_sb[0:1, 6:7])
                nc.sync.dma_start(out_d[u:u + 1, :], zxT[0:1, :])
                if u < HOR - 1:
                    for j in range(NT):
                        ty = ppool.tile([P, 1], F32, name="ty", tag="tp",
                                        bufs=4)
                        nc.tensor.transpose(
                            ty[:, :],
                            zxT[0:1, j * P:(j + 1) * P].bitcast(F32),
                            ident[0:1, 0:1])
                        copy_cast(zd_nat[:, j * ZD:j * ZD + 1], ty[:, :], j)

    _split_multiwait(nc)
    return nc


# ---------------- host-side preprocessing ----------------

def _softplus(x):
    return np.log1p(np.exp(-np.abs(x))) + np.maximum(x, 0.0)


def _host_prep(inp):
    """Edge-weight MLP + row-normalization + dense A^T build + weight
    packing/permutation. Pure per-graph preprocessing (no time loop)."""
    f = np.float32
    row, col = np.asarray(inp["sparse_idx"])
    loc = np.asarray(inp["loc"], f)
    delta = loc[col] - loc[row]
    h1 = np.tanh(delta @ np.asarray(inp["Wk0"], f) + np.asarray(inp["bk0"], f))
    h2 = np.tanh(h1 @ np.asarray(inp["Wk1"], f) + np.asarray(inp["bk1"], f))
    ker = _softplus((h2 @ np.asarray(inp["Wk2"], f)
                     + np.asarray(inp["bk2"], f))[:, 0])
    geo = np.asarray(inp["geodesic"], f)
    w = ker * np.asarray(inp["angle_ratio"], f) * np.exp(-geo * geo)
    denom = np.zeros(N, f)
    np.add.at(denom, row, w)
    w = (w / (denom[row] + np.float32(1e-8))).astype(f)
    at = np.zeros((N, N), f)
    np.add.at(at, (col, row), w)          # at[m, n] = A[n, m]

    # channel permutation: reference z order [feat16|node16|x1|h64]
    # -> ours [feat16|x1|node16|h64]
    px = np.concatenate([np.arange(16), [32], np.arange(16, 32)])
    ph = np.arange(33, 97)

    wg = np.asarray(inp["Wg_e"], f)       # (291, 128)
    wc = np.asarray(inp["Wc_e"], f)       # (291, 64)
    gblocks = [px, ph, np.concatenate([97 + px, 97 + ph]),
               np.concatenate([194 + px, 194 + ph])]
    wge = np.zeros((97, 512), f)
    for i, b in enumerate(gblocks):
        wge[:len(b), i * 128:i * 128 + 128] = wg[b, :]
    cblocks = [px, ph, 97 + px, 97 + ph, 194 + px, 194 + ph]
    wce = np.zeros((64, 384), f)
    for i, b in enumerate(cblocks):
        wce[:len(b), i * 64:i * 64 + 64] = wc[b]

    wgd_r = np.asarray(inp["Wg_d"], f)    # (195, 128)
    wcd_r = np.asarray(inp["Wc_d"], f)    # (195, 64)
    dgblocks = [np.arange(0, 1), np.arange(1, 65), np.arange(65, 130),
                np.arange(130, 195)]
    wgd = np.zeros((65, 512), f)
    for i, b in enumerate(dgblocks):
        wgd[:len(b), i * 128:i * 128 + 128] = wgd_r[b, :]
    dcblocks = [np.arange(0, 1), np.arange(1, 65), np.arange(65, 66),
                np.arange(66, 130), np.arange(130, 131), np.arange(131, 195)]
    wcd = np.zeros((64, 384), f)
    for i, b in enumerate(dcblocks):
        wcd[:len(b), i * 64:i * 64 + 64] = wcd_r[b]

    wfe = np.zeros((2, 17), f)
    wfe[0, 0:16] = np.asarray(inp["W_fe"], f)[0]
    wfe[0, 16] = 1.0
    wfe[1, 0:16] = np.asarray(inp["b_fe"], f)
    bias = np.zeros((64, 8), f)
    bias[:, 0] = np.asarray(inp["bg_e"], f)[0:64]
    bias[:, 1] = np.asarray(inp["bg_e"], f)[64:128]
    bias[:, 2] = np.asarray(inp["bc_e"], f)
    bias[:, 3] = np.asarray(inp["bg_d"], f)[0:64]
    bias[:, 4] = np.asarray(inp["bg_d"], f)[64:128]
    bias[:, 5] = np.asarray(inp["bc_d"], f)
    bias[0, 6] = np.asarray(inp["b_proj"], f)[0]

    node = np.asarray(inp["node_emb"], f)
    bf = ml_dtypes.bfloat16

    def put_f32(blob, r0, nrows, arr):
        v = blob[r0:r0 + nrows].view(np.float32)
        fl = v.reshape(-1)
        a = arr.reshape(-1)
        fl[:a.size] = a

    shared = np.zeros((G_TOT, N), bf)
    shared[G_AT:G_AT + 1024].view(ml_dtypes.float8_e4m3)[:] = (
        at.astype(ml_dtypes.float8_e4m3).reshape(1024, 2 * N))
    shared[G_NODE:G_NODE + EMB] = node.astype(bf).reshape(EMB, N)
    put_f32(shared, G_NODET, 32, np.ascontiguousarray(node.T))
    wge_p = np.zeros((98, 512), f)
    wge_p[:97] = wge
    put_f32(shared, G_WGE, 49, wge_p)
    put_f32(shared, G_WCE, 24, wce)
    wgd_p = np.zeros((66, 512), f)
    wgd_p[:65] = wgd
    put_f32(shared, G_WGD, 33, wgd_p)
    put_f32(shared, G_WCD, 24, wcd)
    frow = np.zeros(1024, f)
    frow[0:17] = wfe[0]
    frow[256:273] = wfe[1]
    frow[512:576] = np.asarray(inp["W_proj"], f)[:, 0]
    put_f32(shared, G_WFE, 1, frow)
    put_f32(shared, G_BIAS, 1, bias)

    xs = np.asarray(inp["inputs"], f)     # (S, B, N, 1)
    in_maps = []
    for b in range(NCORES):
        blob = np.zeros((R_TOT, N), bf)
        if USE_CC:
            blob[0:SHARD] = shared[b * SHARD:(b + 1) * SHARD]
        else:
            blob[0:G_TOT] = shared
        put_f32(blob, R_XS, 24, np.ascontiguousarray(xs[:, b, :, 0]))
        in_maps.append({"blob": blob})
    return in_maps


_NC_CACHE = []


def kernel(**inputs):
    if not _NC_CACHE:
        _NC_CACHE.append(_build())
    nc = _NC_CACHE[0]
    in_maps = _host_prep(inputs)
    res = run_bass_kernel_spmd(nc, in_maps, core_ids=list(range(NCORES)))
    out = np.stack([res.results[b]["out"] for b in range(NCORES)], axis=1)
    return np.ascontiguousarray(out[..., None].astype(np.float32))

